# revision 12
# baseline (speedup 1.0000x reference)
"""GCN encoder (3x GCNConv sharing one normalized adjacency) on 8 TRN2 NeuronCores.

v3:
  - Destination-sharded (edge-cut); per-edge gather of source rows with
    dma_gather spread over all 4 SWDGE queues, 4 calls per batch so all four
    Q7 core pairs stay busy (single-queue descriptor generation is the wall).
  - Self-loop messages pulled out of the edge streams; added in the epilogue
    from SBUF-resident tiles.
  - Scatter-add via TensorE one-hot matmuls accumulating in PSUM per dst tile.
  - Features republished between convs with AllGathers split into chunks that
    are emitted mid-loop, so transfers overlap compute. Gather tables (lo/hi)
    are laid out exactly as the concatenated AllGather outputs; the lo table
    keeps int16-indexable 32768 rows.
  - mu and logstd share one pass: Wc = [W_mu | W_logstd].
"""

import numpy as np

N = 50000
E = 800000
IN = 128
HID = 128
OUT = 64
NCORES = 8
SH = 6272                 # nodes per core (padded)
NPAD = SH * NCORES        # 50176
NT = SH // 128            # 49 dst tiles per core
LHL = 4096                # locals [0, LHL) -> lo table
LHH = SH - LHL            # 2176 locals -> hi table
LOTAB = LHL * NCORES      # 32768 lo-table rows (int16 gather limit)
HITAB = LHH * NCORES      # 17408 hi-table rows
TB = 6                    # dst tiles per gather batch
OHB = 8                   # one-hot chunks generated per DVE op

# AllGather chunking (in per-core local rows). AG1 feeds ftab (conv1 input),
# AG2 feeds hctab (conv2 input); chunk boundaries align with the producing
# loop so each collective fires as soon as its input tiles are written.
AG1_LO = [(0, 4096)]
AG1_HI = [(4096, 6272)]
AG2_LO = [(0, 4096)]
AG2_HI = [(4096, 6272)]

TRACE = False             # test.py sets this for profiling runs
LAST_RESULTS = None       # test.py reads exec_time_ns from here

_CACHE = {}


def _row_map(chunks, base_local):
    """Return (starts, lens, bases) to map local row -> table row."""
    starts = np.array([s for s, _ in chunks], np.int64)
    lens = np.array([e - s for s, e in chunks], np.int64)
    bases = np.concatenate([[0], np.cumsum(lens * NCORES)[:-1]])
    return starts, lens, bases


def _tab_rows(v, lo_chunks, hi_chunks):
    """node id -> (grp, table row) for a table laid out as the concatenation
    of rank-major AllGather chunk outputs."""
    c = v // SH
    l = v % SH
    g = (l >= LHL).astype(np.int64)
    row = np.zeros_like(v)
    for chunks, sel in ((lo_chunks, g == 0), (hi_chunks, g == 1)):
        starts, lens, bases = _row_map(chunks, 0)
        for i in range(len(lens)):
            m = sel & (l >= starts[i]) & (l < starts[i] + lens[i])
            row[m] = bases[i] + c[m] * lens[i] + (l[m] - starts[i])
    return g, row


def _preprocess(edge_index):
    src = np.asarray(edge_index[0]).astype(np.int64)
    dst = np.asarray(edge_index[1]).astype(np.int64)

    # degrees include the appended self-loops (GCNConv default)
    deg = (np.bincount(dst, minlength=N) + 1).astype(np.float32)
    dis = (1.0 / np.sqrt(deg)).astype(np.float32)

    per_core = []
    cnts = np.zeros((NCORES, NT, 2), np.int64)
    for c in range(NCORES):
        m = (dst // SH) == c
        es = src[m]
        ed = dst[m] - c * SH
        t = ed >> 7
        dl = ed & 127
        g1, row1 = _tab_rows(es, AG1_LO, AG1_HI)
        g2, row2 = _tab_rows(es, AG2_LO, AG2_HI)
        assert np.array_equal(g1, g2)
        order = np.lexsort((g1, t))
        row1, row2, t, dl, g = (row1[order], row2[order], t[order],
                                dl[order], g1[order])
        key = t * 2 + g
        bc = np.bincount(key, minlength=NT * 2)
        cnts[c] = bc.reshape(NT, 2)
        per_core.append((row1, row2, t, dl, g, key))

    C = (cnts.max(axis=0) + 127) // 128        # [NT, 2] chunks per (tile, grp)
    KL = int(C[:, 0].sum())
    KH = int(C[:, 1].sum())
    KT = KL + KH

    lo_off = np.concatenate([[0], np.cumsum(C[:, 0])[:-1]])
    hi_off = np.concatenate([[0], np.cumsum(C[:, 1])[:-1]])
    kk_off = np.concatenate([[0], np.cumsum(C.sum(axis=1))[:-1]])

    core_data = []
    for c in range(NCORES):
        row1, row2, t, dl, g, key = per_core[c]
        blk_start = np.concatenate([[0], np.cumsum(cnts[c].reshape(-1))[:-1]])
        rank = np.arange(len(t)) - blk_start[key]
        stream_chunk_off = np.where(g == 0, lo_off[t], hi_off[t])
        pos = stream_chunk_off * 128 + rank
        assert np.array_equal(row1, row2)
        idxs = []
        slo = np.zeros(KL * 128, np.int16)
        shi = np.zeros(KH * 128, np.int16)
        slo[pos[g == 0]] = row1[g == 0].astype(np.int16)
        shi[pos[g == 1]] = row1[g == 1].astype(np.int16)
        idxs.append(np.tile(slo.reshape(-1, 16).T, (8, 1)))
        idxs.append(np.tile(shi.reshape(-1, 16).T, (8, 1)))
        kk = np.where(g == 0, kk_off[t], kk_off[t] + C[t, 0]) + rank // 128
        dest = np.full(KT * 128, 255.0, np.float16)
        dest[kk * 128 + rank % 128] = dl.astype(np.float16)
        destT = np.ascontiguousarray(dest.reshape(KT, 128).T)  # [128, KT]
        core_data.append((idxs, destT))

    batches = []
    t0 = 0
    sizes = [2, 4] + [TB] * NT
    for sz in sizes:
        if t0 >= NT:
            break
        batches.append((t0, min(t0 + sz, NT)))
        t0 = min(t0 + sz, NT)
    meta = dict(C=C, KL=KL, KH=KH, KT=KT,
                lo_off=lo_off, hi_off=hi_off, kk_off=kk_off, batches=batches)
    return dis, core_data, meta


def _build_nc(meta, has_b1, has_bc):
    import concourse.bass as bass
    import concourse.bacc as bacc
    import concourse.mybir as mybir
    import concourse.tile as tile
    from concourse import library_config

    C = meta["C"]
    KL, KH, KT = meta["KL"], meta["KH"], meta["KT"]
    lo_off, hi_off, kk_off = meta["lo_off"], meta["hi_off"], meta["kk_off"]
    batches = meta["batches"]

    f16 = mybir.dt.float16
    f32 = mybir.dt.float32
    i16 = mybir.dt.int16
    eq = mybir.AluOpType.is_equal
    mult = mybir.AluOpType.mult
    add = mybir.AluOpType.add
    Relu = mybir.ActivationFunctionType.Relu
    Copy = mybir.ActivationFunctionType.Copy

    nc = bacc.Bacc("TRN2", target_bir_lowering=False, debug=False,
                   enable_asserts=False, num_devices=NCORES,
                   num_swdge_queues=4)

    xTs = nc.dram_tensor("xTs", [128, SH], f16, kind="ExternalInput")
    W1d = nc.dram_tensor("W1d", [128, 128], f16, kind="ExternalInput")
    Wcd = nc.dram_tensor("Wcd", [128, 128], f16, kind="ExternalInput")
    dis2Td = nc.dram_tensor("dis2Td", [128, NT], f32, kind="ExternalInput")
    disTd = nc.dram_tensor("disTd", [128, NT], f32, kind="ExternalInput")
    iotad = nc.dram_tensor("iotad", [128, OHB * 128], f16, kind="ExternalInput")
    identd = nc.dram_tensor("identd", [128, 128], f16, kind="ExternalInput")
    idxl1d = nc.dram_tensor("idxl1d", [128, KL * 8], i16, kind="ExternalInput")
    idxh1d = nc.dram_tensor("idxh1d", [128, KH * 8], i16, kind="ExternalInput")
    destTd = nc.dram_tensor("destTd", [128, KT], f16, kind="ExternalInput")
    if has_b1:
        b1rd = nc.dram_tensor("b1rd", [128, 128], f32, kind="ExternalInput")
    if has_bc:
        bcrd = nc.dram_tensor("bcrd", [128, 128], f32, kind="ExternalInput")
    out_ml = nc.dram_tensor("out_ml", [SH, 128], f32, kind="ExternalOutput")

    qcnt = [0]

    def next_q():
        q = qcnt[0] % 4
        qcnt[0] += 1
        return q

    with tile.TileContext(nc) as tc:
        with (
            tc.tile_pool(name="consts", bufs=1) as cpool,
            tc.tile_pool(name="keep", bufs=1) as kpool,
            tc.tile_pool(name="xin", bufs=3) as xpool,
            tc.tile_pool(name="work", bufs=4) as wpool,
            tc.tile_pool(name="oh", bufs=6) as ohpool,
            tc.tile_pool(name="glo", bufs=3) as gpool_lo,
            tc.tile_pool(name="ghi", bufs=3) as gpool_hi,
            tc.tile_pool(name="psA", bufs=2, space="PSUM") as psA,
            tc.tile_pool(name="psB", bufs=3, space="PSUM") as psB,
            tc.tile_pool(name="psT", bufs=1, space="PSUM") as psT,
            tc.tile_pool(name="psH", bufs=1, space="PSUM") as psH,
            tc.tile_pool(name="dram", bufs=1, space="DRAM") as dpool,
        ):
            nc.gpsimd.load_library(library_config.mlp)

            xsb = cpool.tile([128, SH], f16, tag="xsb")
            W1sb = cpool.tile([128, 128], f16, tag="W1sb")
            Wcsb = cpool.tile([128, 128], f16, tag="Wcsb")
            dis2sb = cpool.tile([128, NT], f32, tag="dis2sb")
            dissb = cpool.tile([128, NT], f32, tag="dissb")
            iotasb = cpool.tile([128, OHB * 128], f16, tag="iotasb")
            identsb = cpool.tile([128, 128], f16, tag="identsb")
            idxl1sb = cpool.tile([128, KL * 8], i16, tag="idxl1sb")
            idxh1sb = cpool.tile([128, KH * 8], i16, tag="idxh1sb")
            destTsb = cpool.tile([128, KT], f16, tag="destTsb")

            nc.sync.dma_start(xsb[:], xTs.ap())
            nc.sync.dma_start(W1sb[:], W1d.ap())
            nc.sync.dma_start(Wcsb[:], Wcd.ap())
            nc.sync.dma_start(dis2sb[:], dis2Td.ap())
            nc.sync.dma_start(dissb[:], disTd.ap())
            nc.sync.dma_start(iotasb[:], iotad.ap())
            nc.sync.dma_start(identsb[:], identd.ap())
            nc.sync.dma_start(idxl1sb[:], idxl1d.ap())
            nc.sync.dma_start(idxh1sb[:], idxh1d.ap())
            nc.sync.dma_start(destTsb[:], destTd.ap())
            if has_b1:
                b1sb = cpool.tile([128, 128], f32, tag="b1sb")
                nc.sync.dma_start(b1sb[:], b1rd.ap())
            if has_bc:
                bcsb = cpool.tile([128, 128], f32, tag="bcsb")
                nc.sync.dma_start(bcsb[:], bcrd.ap())

            gkeep = kpool.tile([128, NT, 128], f16, tag="gkeep")
            hcdkeep = kpool.tile([128, NT, 128], f16, tag="hcdkeep")

            h0sa = dpool.tile([LHL, 128], f16, tag="h0sa")
            h0sb = dpool.tile([LHH, 128], f16, tag="h0sb")
            hcsa = dpool.tile([LHL, 128], f16, tag="hcsa")
            hcsb = dpool.tile([LHH, 128], f16, tag="hcsb")
            ftab_lo = dpool.tile([LOTAB, 128], f16, tag="ftab_lo",
                                 addr_space="Shared")
            ftab_hi = dpool.tile([HITAB, 128], f16, tag="ftab_hi",
                                 addr_space="Shared")
            hctab_lo = dpool.tile([LOTAB, 128], f16, tag="hctab_lo",
                                  addr_space="Shared")
            hctab_hi = dpool.tile([HITAB, 128], f16, tag="hctab_hi",
                                  addr_space="Shared")

            def emit_ag(chunks, src_lo, src_hi, out_lo, out_hi, done_local,
                        fired):
                """Emit any not-yet-fired AllGather chunks whose input rows are
                all written (tiles complete in order; done_local = completed
                local row count)."""
                for (kind, s, e) in chunks:
                    if e > done_local or (kind, s, e) in fired:
                        continue
                    fired.add((kind, s, e))
                    if kind == 0:
                        ins, outs, base = src_lo, out_lo, 0
                        boff = sum((ee - ss) * NCORES for (kk, ss, ee) in chunks
                                   if kk == 0 and ee <= s)
                        ins_ap = ins[s:e, :].opt()
                        outs_ap = outs[boff:boff + (e - s) * NCORES, :].opt()
                    else:
                        ins, outs = src_hi, out_hi
                        boff = sum((ee - ss) * NCORES for (kk, ss, ee) in chunks
                                   if kk == 1 and ee <= s)
                        ins_ap = ins[s - LHL:e - LHL, :].opt()
                        outs_ap = outs[boff:boff + (e - s) * NCORES, :].opt()
                    nc.gpsimd.collective_compute(
                        "AllGather", mybir.AluOpType.bypass,
                        replica_groups=[list(range(NCORES))],
                        ins=[ins_ap], outs=[outs_ap])

            ag1_chunks = ([(0, s, e) for (s, e) in AG1_LO] +
                          [(1, s, e) for (s, e) in AG1_HI])
            ag2_chunks = ([(0, s, e) for (s, e) in AG2_LO] +
                          [(1, s, e) for (s, e) in AG2_HI])
            ag1_fired = set()
            ag2_fired = set()

            # ---- Phase A: g = (x*dis)@W1 rows of my shard ----
            for t in range(NT):
                ps = psA.tile([128, 128], f32, tag="psA")
                nc.tensor.matmul(ps[:], xsb[:, t * 128:(t + 1) * 128],
                                 W1sb[:], start=True, stop=True)
                ht = xpool.tile([128, 128], f16, tag="ht")
                nc.scalar.copy(ht[:], ps[:])
                scl = dissb if has_b1 else dis2sb
                nc.scalar.activation(gkeep[:, t, :], ps[:], Copy,
                                     scale=scl[:, t:t + 1])
                if t < 32:
                    nc.sync.dma_start(h0sa[t * 128:(t + 1) * 128, :], ht[:])
                else:
                    tt = t - 32
                    nc.sync.dma_start(h0sb[tt * 128:(tt + 1) * 128, :], ht[:])
                emit_ag(ag1_chunks, h0sa, h0sb, ftab_lo, ftab_hi,
                        (t + 1) * 128 if t < 32 else LHL + (t - 31) * 128,
                        ag1_fired)

            def conv_pass(tab_lo, tab_hi, idxlo, idxhi, is_conv1):
                for bi, (t0, t1) in enumerate(batches):
                    cl = int(C[t0:t1, 0].sum())
                    ch = int(C[t0:t1, 1].sum())
                    glo = ghi = None
                    if cl:
                        glo = gpool_lo.tile([128, cl, 128], f16, tag="glo")
                    if ch:
                        ghi = gpool_hi.tile([128, ch, 128], f16, tag="ghi")
                    jobs = []
                    if cl:
                        ca = (cl + 1) // 2
                        jobs.append((glo, 0, ca, tab_lo, LOTAB, idxlo, lo_off[t0]))
                        if cl - ca:
                            jobs.append((glo, ca, cl - ca, tab_lo, LOTAB, idxlo,
                                         lo_off[t0]))
                    if ch:
                        ca = (ch + 1) // 2
                        jobs.append((ghi, 0, ca, tab_hi, HITAB, idxhi, hi_off[t0]))
                        if ch - ca:
                            jobs.append((ghi, ca, ch - ca, tab_hi, HITAB, idxhi,
                                         hi_off[t0]))
                    for (gt, o, n, tab, trows, idxs, goff) in jobs:
                        i0 = (int(goff) + o) * 8
                        nc.gpsimd.dma_gather(
                            gt[:, o:o + n, :], tab[0:trows, :],
                            idxs[:, i0:i0 + n * 8],
                            num_idxs=n * 128, num_idxs_reg=n * 128,
                            elem_size=128, single_packet=False,
                            queue_num=next_q())

                    for t in range(t0, t1):
                        nch = int(C[t, 0] + C[t, 1])
                        kk0 = int(kk_off[t])
                        ohs = []
                        j = 0
                        while j < nch:
                            nb = min(OHB, nch - j)
                            oh = ohpool.tile([128, nb, 128], f16, tag="oh")
                            nc.vector.tensor_tensor(
                                oh[:],
                                iotasb[:, 0:nb * 128].rearrange(
                                    "p (c e) -> p c e", e=128),
                                destTsb[:, kk0 + j:kk0 + j + nb].broadcast_to(
                                    [128, nb, 128]),
                                eq)
                            ohs.append((j, nb, oh))
                            j += nb

                        def oh_at(k):
                            for (jj, nb, oh) in ohs:
                                if jj <= k < jj + nb:
                                    return oh[:, k - jj, :]
                            raise AssertionError

                        ps = psB.tile([128, 128], f32, tag="psB")
                        k = 0
                        for j2 in range(int(C[t, 0])):
                            src = glo[:, int(lo_off[t] - lo_off[t0]) + j2, :]
                            nc.tensor.matmul(ps[:], oh_at(k), src,
                                             start=(k == 0), stop=(k == nch - 1),
                                             skip_group_check=True)
                            k += 1
                        for j2 in range(int(C[t, 1])):
                            src = ghi[:, int(hi_off[t] - hi_off[t0]) + j2, :]
                            nc.tensor.matmul(ps[:], oh_at(k), src,
                                             start=(k == 0), stop=(k == nch - 1),
                                             skip_group_check=True)
                            k += 1

                        if is_conv1:
                            # hs = dis * relu(dis*(ps + g) [+ b1])
                            if has_b1:
                                u0 = wpool.tile([128, 128], f32, tag="u0")
                                nc.vector.scalar_tensor_tensor(
                                    u0[:], ps[:], dissb[:, t:t + 1],
                                    gkeep[:, t, :], mult, add)
                                u1 = wpool.tile([128, 128], f32, tag="u1")
                                nc.vector.tensor_tensor(u1[:], u0[:], b1sb[:], add)
                                hs = wpool.tile([128, 128], f16, tag="hs")
                                nc.scalar.activation(hs[:], u1[:], Relu,
                                                     scale=dissb[:, t:t + 1])
                            else:
                                u0 = wpool.tile([128, 128], f16, tag="u0")
                                nc.vector.scalar_tensor_tensor(
                                    u0[:], ps[:], dis2sb[:, t:t + 1],
                                    gkeep[:, t, :], mult, add)
                                hs = wpool.tile([128, 128], f16, tag="hs")
                                nc.scalar.activation(hs[:], u0[:], Relu)
                            pst = psT.tile([128, 128], f16, tag="psT")
                            nc.tensor.transpose(pst[:], hs[:], identsb[:])
                            hsT = wpool.tile([128, 128], f16, tag="hsT")
                            nc.scalar.copy(hsT[:], pst[:])
                            psh = psH.tile([128, 128], f32, tag="psH")
                            nc.tensor.matmul(psh[:], hsT[:], Wcsb[:],
                                             start=True, stop=True,
                                             skip_group_check=True)
                            hct = wpool.tile([128, 128], f16, tag="hct")
                            nc.scalar.copy(hct[:], psh[:])
                            nc.scalar.activation(hcdkeep[:, t, :], psh[:], Copy,
                                                 scale=dissb[:, t:t + 1])
                            if t < 32:
                                nc.sync.dma_start(
                                    hcsa[t * 128:(t + 1) * 128, :], hct[:])
                            else:
                                tt = t - 32
                                nc.sync.dma_start(
                                    hcsb[tt * 128:(tt + 1) * 128, :], hct[:])
                        else:
                            ot = wpool.tile([128, 128], f32, tag="ot")
                            nc.vector.scalar_tensor_tensor(
                                ot[:], ps[:], dissb[:, t:t + 1],
                                hcdkeep[:, t, :], mult, add)
                            if has_bc:
                                nc.vector.tensor_tensor(ot[:], ot[:], bcsb[:], add)
                            nc.sync.dma_start(
                                out_ml.ap()[t * 128:(t + 1) * 128, :], ot[:])
                    if is_conv1:
                        emit_ag(ag2_chunks, hcsa, hcsb, hctab_lo, hctab_hi,
                                t1 * 128 if t1 <= 32 else LHL + (t1 - 32) * 128,
                                ag2_fired)

            conv_pass(ftab_lo, ftab_hi, idxl1sb, idxh1sb, True)
            conv_pass(hctab_lo, hctab_hi, idxl1sb, idxh1sb, False)

    nc.compile()
    return nc


def kernel(x, edge_index, W1, b1, W_mu, b_mu, W_logstd, b_logstd):
    global LAST_RESULTS
    from concourse.bass_utils import run_bass_kernel_spmd

    x = np.asarray(x, dtype=np.float32)
    W1 = np.asarray(W1, dtype=np.float32)
    b1 = np.asarray(b1, dtype=np.float32)
    W_mu = np.asarray(W_mu, dtype=np.float32)
    b_mu = np.asarray(b_mu, dtype=np.float32)
    W_logstd = np.asarray(W_logstd, dtype=np.float32)
    b_logstd = np.asarray(b_logstd, dtype=np.float32)

    has_b1 = bool(np.any(b1 != 0.0))
    bc = np.concatenate([b_mu, b_logstd])
    has_bc = bool(np.any(bc != 0.0))

    ek = np.asarray(edge_index).tobytes()
    key = (ek[:64] + ek[-64:], len(ek), has_b1, has_bc)
    cached = _CACHE.get("k")
    if cached is not None and cached[0] == key:
        _, dis, core_data, meta, nc = cached
    else:
        dis, core_data, meta = _preprocess(edge_index)
        nc = _build_nc(meta, has_b1, has_bc)
        _CACHE["k"] = (key, dis, core_data, meta, nc)

    x2T = np.zeros((IN, NPAD), np.float16)
    x2T[:, :N] = (x * dis[:, None]).T.astype(np.float16)
    W1h = W1.astype(np.float16)
    Wch = np.concatenate([W_mu, W_logstd], axis=1).astype(np.float16)
    disP = np.zeros(NPAD, np.float32)
    disP[:N] = dis
    iota = np.tile(np.arange(128, dtype=np.float16)[None, :], (128, OHB))
    ident = np.eye(128, dtype=np.float16)

    in_maps = []
    for c in range(NCORES):
        idxs, destT = core_data[c]
        disSh = disP[c * SH:(c + 1) * SH].reshape(NT, 128).T  # [128, NT]
        im = {
            "xTs": np.ascontiguousarray(x2T[:, c * SH:(c + 1) * SH]),
            "W1d": W1h, "Wcd": Wch,
            "dis2Td": np.ascontiguousarray((disSh * disSh).astype(np.float32)),
            "disTd": np.ascontiguousarray(disSh.astype(np.float32)),
            "iotad": np.ascontiguousarray(iota),
            "identd": ident,
            "idxl1d": idxs[0], "idxh1d": idxs[1],
            "destTd": destT,
        }
        if has_b1:
            im["b1rd"] = np.tile(b1[None, :], (128, 1)).astype(np.float32)
        if has_bc:
            im["bcrd"] = np.tile(bc[None, :], (128, 1)).astype(np.float32)
        in_maps.append(im)

    res = run_bass_kernel_spmd(nc, in_maps, core_ids=list(range(NCORES)),
                               trace=TRACE)
    LAST_RESULTS = res
    full = np.concatenate([res.results[c]["out_ml"] for c in range(NCORES)],
                          axis=0)[:N]
    mu = np.ascontiguousarray(full[:, :OUT])
    logstd = np.ascontiguousarray(full[:, OUT:])
    return (mu, logstd)


# revision 13
# speedup vs baseline: 1.0070x; 1.0070x over previous
"""GCN encoder (3x GCNConv sharing one normalized adjacency) on 8 TRN2 NeuronCores.

v3:
  - Destination-sharded (edge-cut); per-edge gather of source rows with
    dma_gather spread over all 4 SWDGE queues, 4 calls per batch so all four
    Q7 core pairs stay busy (single-queue descriptor generation is the wall).
  - Self-loop messages pulled out of the edge streams; added in the epilogue
    from SBUF-resident tiles.
  - Scatter-add via TensorE one-hot matmuls accumulating in PSUM per dst tile.
  - Features republished between convs with AllGathers split into chunks that
    are emitted mid-loop, so transfers overlap compute. Gather tables (lo/hi)
    are laid out exactly as the concatenated AllGather outputs; the lo table
    keeps int16-indexable 32768 rows.
  - mu and logstd share one pass: Wc = [W_mu | W_logstd].
"""

import numpy as np

N = 50000
E = 800000
IN = 128
HID = 128
OUT = 64
NCORES = 8
SH = 6272                 # nodes per core (padded)
NPAD = SH * NCORES        # 50176
NT = SH // 128            # 49 dst tiles per core
LHL = 4096                # locals [0, LHL) -> lo table
LHH = SH - LHL            # 2176 locals -> hi table
LOTAB = LHL * NCORES      # 32768 lo-table rows (int16 gather limit)
HITAB = LHH * NCORES      # 17408 hi-table rows
TB = 6                    # dst tiles per gather batch
OHB = 8                   # one-hot chunks generated per DVE op

# AllGather chunking (in per-core local rows). AG1 feeds ftab (conv1 input),
# AG2 feeds hctab (conv2 input); chunk boundaries align with the producing
# loop so each collective fires as soon as its input tiles are written.
AG1_LO = [(0, 4096)]
AG1_HI = [(4096, 6272)]
AG2_LO = [(0, 4096)]
AG2_HI = [(4096, 6272)]

TRACE = False             # test.py sets this for profiling runs
LAST_RESULTS = None       # test.py reads exec_time_ns from here

_CACHE = {}


def _row_map(chunks, base_local):
    """Return (starts, lens, bases) to map local row -> table row."""
    starts = np.array([s for s, _ in chunks], np.int64)
    lens = np.array([e - s for s, e in chunks], np.int64)
    bases = np.concatenate([[0], np.cumsum(lens * NCORES)[:-1]])
    return starts, lens, bases


def _tab_rows(v, lo_chunks, hi_chunks):
    """node id -> (grp, table row) for a table laid out as the concatenation
    of rank-major AllGather chunk outputs."""
    c = v // SH
    l = v % SH
    g = (l >= LHL).astype(np.int64)
    row = np.zeros_like(v)
    for chunks, sel in ((lo_chunks, g == 0), (hi_chunks, g == 1)):
        starts, lens, bases = _row_map(chunks, 0)
        for i in range(len(lens)):
            m = sel & (l >= starts[i]) & (l < starts[i] + lens[i])
            row[m] = bases[i] + c[m] * lens[i] + (l[m] - starts[i])
    return g, row


def _preprocess(edge_index):
    src = np.asarray(edge_index[0]).astype(np.int64)
    dst = np.asarray(edge_index[1]).astype(np.int64)

    # degrees include the appended self-loops (GCNConv default)
    deg = (np.bincount(dst, minlength=N) + 1).astype(np.float32)
    dis = (1.0 / np.sqrt(deg)).astype(np.float32)

    per_core = []
    cnts = np.zeros((NCORES, NT, 2), np.int64)
    for c in range(NCORES):
        m = (dst // SH) == c
        es = src[m]
        ed = dst[m] - c * SH
        t = ed >> 7
        dl = ed & 127
        g1, row1 = _tab_rows(es, AG1_LO, AG1_HI)
        g2, row2 = _tab_rows(es, AG2_LO, AG2_HI)
        assert np.array_equal(g1, g2)
        order = np.lexsort((g1, t))
        row1, row2, t, dl, g = (row1[order], row2[order], t[order],
                                dl[order], g1[order])
        key = t * 2 + g
        bc = np.bincount(key, minlength=NT * 2)
        cnts[c] = bc.reshape(NT, 2)
        per_core.append((row1, row2, t, dl, g, key))

    C = (cnts.max(axis=0) + 127) // 128        # [NT, 2] chunks per (tile, grp)
    KL = int(C[:, 0].sum())
    KH = int(C[:, 1].sum())
    KT = KL + KH

    lo_off = np.concatenate([[0], np.cumsum(C[:, 0])[:-1]])
    hi_off = np.concatenate([[0], np.cumsum(C[:, 1])[:-1]])
    kk_off = np.concatenate([[0], np.cumsum(C.sum(axis=1))[:-1]])

    core_data = []
    for c in range(NCORES):
        row1, row2, t, dl, g, key = per_core[c]
        blk_start = np.concatenate([[0], np.cumsum(cnts[c].reshape(-1))[:-1]])
        rank = np.arange(len(t)) - blk_start[key]
        stream_chunk_off = np.where(g == 0, lo_off[t], hi_off[t])
        pos = stream_chunk_off * 128 + rank
        assert np.array_equal(row1, row2)
        idxs = []
        slo = np.zeros(KL * 128, np.int16)
        shi = np.zeros(KH * 128, np.int16)
        slo[pos[g == 0]] = row1[g == 0].astype(np.int16)
        shi[pos[g == 1]] = row1[g == 1].astype(np.int16)
        idxs.append(np.tile(slo.reshape(-1, 16).T, (8, 1)))
        idxs.append(np.tile(shi.reshape(-1, 16).T, (8, 1)))
        kk = np.where(g == 0, kk_off[t], kk_off[t] + C[t, 0]) + rank // 128
        dest = np.full(KT * 128, 255.0, np.float16)
        dest[kk * 128 + rank % 128] = dl.astype(np.float16)
        destT = np.ascontiguousarray(dest.reshape(KT, 128).T)  # [128, KT]
        core_data.append((idxs, destT))

    batches = []
    t0 = 0
    sizes = [2, 4] + [TB] * NT
    for sz in sizes:
        if t0 >= NT:
            break
        batches.append((t0, min(t0 + sz, NT)))
        t0 = min(t0 + sz, NT)
    meta = dict(C=C, KL=KL, KH=KH, KT=KT,
                lo_off=lo_off, hi_off=hi_off, kk_off=kk_off, batches=batches)
    return dis, core_data, meta


def _build_nc(meta, has_b1, has_bc):
    import concourse.bass as bass
    import concourse.bacc as bacc
    import concourse.mybir as mybir
    import concourse.tile as tile
    from concourse import library_config

    C = meta["C"]
    KL, KH, KT = meta["KL"], meta["KH"], meta["KT"]
    lo_off, hi_off, kk_off = meta["lo_off"], meta["hi_off"], meta["kk_off"]
    batches = meta["batches"]

    f16 = mybir.dt.float16
    f32 = mybir.dt.float32
    i16 = mybir.dt.int16
    eq = mybir.AluOpType.is_equal
    mult = mybir.AluOpType.mult
    add = mybir.AluOpType.add
    Relu = mybir.ActivationFunctionType.Relu
    Copy = mybir.ActivationFunctionType.Copy

    nc = bacc.Bacc("TRN2", target_bir_lowering=False, debug=False,
                   enable_asserts=False, num_devices=NCORES,
                   num_swdge_queues=4)

    xTs = nc.dram_tensor("xTs", [128, SH], f16, kind="ExternalInput")
    W1d = nc.dram_tensor("W1d", [128, 128], f16, kind="ExternalInput")
    Wcd = nc.dram_tensor("Wcd", [128, 128], f16, kind="ExternalInput")
    dis2Td = nc.dram_tensor("dis2Td", [128, NT], f32, kind="ExternalInput")
    disTd = nc.dram_tensor("disTd", [128, NT], f32, kind="ExternalInput")
    iotad = nc.dram_tensor("iotad", [128, OHB * 128], f16, kind="ExternalInput")
    identd = nc.dram_tensor("identd", [128, 128], f16, kind="ExternalInput")
    idxl1d = nc.dram_tensor("idxl1d", [128, KL * 8], i16, kind="ExternalInput")
    idxh1d = nc.dram_tensor("idxh1d", [128, KH * 8], i16, kind="ExternalInput")
    destTd = nc.dram_tensor("destTd", [128, KT], f16, kind="ExternalInput")
    if has_b1:
        b1rd = nc.dram_tensor("b1rd", [128, 128], f32, kind="ExternalInput")
    if has_bc:
        bcrd = nc.dram_tensor("bcrd", [128, 128], f32, kind="ExternalInput")
    out_ml = nc.dram_tensor("out_ml", [SH, 128], f32, kind="ExternalOutput")

    qcnt = [0]

    def next_q():
        q = qcnt[0] % 4
        qcnt[0] += 1
        return q

    with tile.TileContext(nc) as tc:
        with (
            tc.tile_pool(name="consts", bufs=1) as cpool,
            tc.tile_pool(name="keep", bufs=1) as kpool,
            tc.tile_pool(name="xin", bufs=3) as xpool,
            tc.tile_pool(name="work", bufs=4) as wpool,
            tc.tile_pool(name="oh", bufs=6) as ohpool,
            tc.tile_pool(name="glo", bufs=3) as gpool_lo,
            tc.tile_pool(name="ghi", bufs=3) as gpool_hi,
            tc.tile_pool(name="psA", bufs=2, space="PSUM") as psA,
            tc.tile_pool(name="psB", bufs=4, space="PSUM") as psB,
            tc.tile_pool(name="psT", bufs=1, space="PSUM") as psT,
            tc.tile_pool(name="psH", bufs=1, space="PSUM") as psH,
            tc.tile_pool(name="dram", bufs=1, space="DRAM") as dpool,
        ):
            nc.gpsimd.load_library(library_config.mlp)

            xsb = cpool.tile([128, SH], f16, tag="xsb")
            W1sb = cpool.tile([128, 128], f16, tag="W1sb")
            Wcsb = cpool.tile([128, 128], f16, tag="Wcsb")
            dis2sb = cpool.tile([128, NT], f32, tag="dis2sb")
            dissb = cpool.tile([128, NT], f32, tag="dissb")
            iotasb = cpool.tile([128, OHB * 128], f16, tag="iotasb")
            identsb = cpool.tile([128, 128], f16, tag="identsb")
            idxl1sb = cpool.tile([128, KL * 8], i16, tag="idxl1sb")
            idxh1sb = cpool.tile([128, KH * 8], i16, tag="idxh1sb")
            destTsb = cpool.tile([128, KT], f16, tag="destTsb")

            nc.sync.dma_start(xsb[:], xTs.ap())
            nc.sync.dma_start(W1sb[:], W1d.ap())
            nc.sync.dma_start(Wcsb[:], Wcd.ap())
            nc.sync.dma_start(dis2sb[:], dis2Td.ap())
            nc.sync.dma_start(dissb[:], disTd.ap())
            nc.sync.dma_start(iotasb[:], iotad.ap())
            nc.sync.dma_start(identsb[:], identd.ap())
            nc.sync.dma_start(idxl1sb[:], idxl1d.ap())
            nc.sync.dma_start(idxh1sb[:], idxh1d.ap())
            nc.sync.dma_start(destTsb[:], destTd.ap())
            if has_b1:
                b1sb = cpool.tile([128, 128], f32, tag="b1sb")
                nc.sync.dma_start(b1sb[:], b1rd.ap())
            if has_bc:
                bcsb = cpool.tile([128, 128], f32, tag="bcsb")
                nc.sync.dma_start(bcsb[:], bcrd.ap())

            gkeep = kpool.tile([128, NT, 128], f16, tag="gkeep")
            hcdkeep = kpool.tile([128, NT, 128], f16, tag="hcdkeep")

            h0sa = dpool.tile([LHL, 128], f16, tag="h0sa")
            h0sb = dpool.tile([LHH, 128], f16, tag="h0sb")
            hcsa = dpool.tile([LHL, 128], f16, tag="hcsa")
            hcsb = dpool.tile([LHH, 128], f16, tag="hcsb")
            ftab_lo = dpool.tile([LOTAB, 128], f16, tag="ftab_lo",
                                 addr_space="Shared")
            ftab_hi = dpool.tile([HITAB, 128], f16, tag="ftab_hi",
                                 addr_space="Shared")
            hctab_lo = dpool.tile([LOTAB, 128], f16, tag="hctab_lo",
                                  addr_space="Shared")
            hctab_hi = dpool.tile([HITAB, 128], f16, tag="hctab_hi",
                                  addr_space="Shared")

            def emit_ag(chunks, src_lo, src_hi, out_lo, out_hi, done_local,
                        fired):
                """Emit any not-yet-fired AllGather chunks whose input rows are
                all written (tiles complete in order; done_local = completed
                local row count)."""
                for (kind, s, e) in chunks:
                    if e > done_local or (kind, s, e) in fired:
                        continue
                    fired.add((kind, s, e))
                    if kind == 0:
                        ins, outs, base = src_lo, out_lo, 0
                        boff = sum((ee - ss) * NCORES for (kk, ss, ee) in chunks
                                   if kk == 0 and ee <= s)
                        ins_ap = ins[s:e, :].opt()
                        outs_ap = outs[boff:boff + (e - s) * NCORES, :].opt()
                    else:
                        ins, outs = src_hi, out_hi
                        boff = sum((ee - ss) * NCORES for (kk, ss, ee) in chunks
                                   if kk == 1 and ee <= s)
                        ins_ap = ins[s - LHL:e - LHL, :].opt()
                        outs_ap = outs[boff:boff + (e - s) * NCORES, :].opt()
                    nc.gpsimd.collective_compute(
                        "AllGather", mybir.AluOpType.bypass,
                        replica_groups=[list(range(NCORES))],
                        ins=[ins_ap], outs=[outs_ap])

            ag1_chunks = ([(0, s, e) for (s, e) in AG1_LO] +
                          [(1, s, e) for (s, e) in AG1_HI])
            ag2_chunks = ([(0, s, e) for (s, e) in AG2_LO] +
                          [(1, s, e) for (s, e) in AG2_HI])
            ag1_fired = set()
            ag2_fired = set()

            # ---- Phase A: g = (x*dis)@W1 rows of my shard ----
            for t in range(NT):
                ps = psA.tile([128, 128], f32, tag="psA")
                nc.tensor.matmul(ps[:], xsb[:, t * 128:(t + 1) * 128],
                                 W1sb[:], start=True, stop=True)
                ht = xpool.tile([128, 128], f16, tag="ht")
                nc.scalar.copy(ht[:], ps[:])
                scl = dissb if has_b1 else dis2sb
                nc.scalar.activation(gkeep[:, t, :], ps[:], Copy,
                                     scale=scl[:, t:t + 1])
                if t < 32:
                    nc.sync.dma_start(h0sa[t * 128:(t + 1) * 128, :], ht[:])
                else:
                    tt = t - 32
                    nc.sync.dma_start(h0sb[tt * 128:(tt + 1) * 128, :], ht[:])
                emit_ag(ag1_chunks, h0sa, h0sb, ftab_lo, ftab_hi,
                        (t + 1) * 128 if t < 32 else LHL + (t - 31) * 128,
                        ag1_fired)

            def make_conv(tab_lo, tab_hi, idxlo, idxhi, is_conv1):
                lo_cache = {}
                hi_cache = {}

                def run_jobs(jobs):
                    for (gt, o, n, tab, trows, idxs, goff) in jobs:
                        i0 = (int(goff) + o) * 8
                        nc.gpsimd.dma_gather(
                            gt[:, o:o + n, :], tab[0:trows, :],
                            idxs[:, i0:i0 + n * 8],
                            num_idxs=n * 128, num_idxs_reg=n * 128,
                            elem_size=128, single_packet=False,
                            queue_num=next_q())

                def glo_for(bi):
                    if bi in lo_cache:
                        return lo_cache[bi]
                    t0, t1 = batches[bi]
                    cl = int(C[t0:t1, 0].sum())
                    glo = None
                    if cl:
                        glo = gpool_lo.tile([128, cl, 128], f16, tag="glo",
                                            name=f"glo{int(is_conv1)}_{bi}")
                        ca = (cl + 1) // 2
                        jobs = [(glo, 0, ca, tab_lo, LOTAB, idxlo, lo_off[t0])]
                        if cl - ca:
                            jobs.append((glo, ca, cl - ca, tab_lo, LOTAB,
                                         idxlo, lo_off[t0]))
                        run_jobs(jobs)
                    lo_cache[bi] = glo
                    return glo

                def ghi_for(bi):
                    if bi in hi_cache:
                        return hi_cache[bi]
                    t0, t1 = batches[bi]
                    ch = int(C[t0:t1, 1].sum())
                    ghi = None
                    if ch:
                        ghi = gpool_hi.tile([128, ch, 128], f16, tag="ghi",
                                            name=f"ghi{int(is_conv1)}_{bi}")
                        ca = (ch + 1) // 2
                        jobs = [(ghi, 0, ca, tab_hi, HITAB, idxhi, hi_off[t0])]
                        if ch - ca:
                            jobs.append((ghi, ca, ch - ca, tab_hi, HITAB,
                                         idxhi, hi_off[t0]))
                        run_jobs(jobs)
                    hi_cache[bi] = ghi
                    return ghi

                return glo_for, ghi_for

            def conv_pass(tab_lo, tab_hi, idxlo, idxhi, is_conv1,
                          emitters=None):
                if emitters is None:
                    emitters = make_conv(tab_lo, tab_hi, idxlo, idxhi,
                                         is_conv1)
                glo_for, ghi_for = emitters
                for bi, (t0, t1) in enumerate(batches):
                    glo = glo_for(bi)
                    ghi = ghi_for(bi)

                    for t in range(t0, t1):
                        nch = int(C[t, 0] + C[t, 1])
                        kk0 = int(kk_off[t])
                        ohs = []
                        j = 0
                        while j < nch:
                            nb = min(OHB, nch - j)
                            oh = ohpool.tile([128, nb, 128], f16, tag="oh")
                            nc.vector.tensor_tensor(
                                oh[:],
                                iotasb[:, 0:nb * 128].rearrange(
                                    "p (c e) -> p c e", e=128),
                                destTsb[:, kk0 + j:kk0 + j + nb].broadcast_to(
                                    [128, nb, 128]),
                                eq)
                            ohs.append((j, nb, oh))
                            j += nb

                        def oh_at(k):
                            for (jj, nb, oh) in ohs:
                                if jj <= k < jj + nb:
                                    return oh[:, k - jj, :]
                            raise AssertionError

                        ps = psB.tile([128, 128], f32, tag="psB")
                        k = 0
                        for j2 in range(int(C[t, 0])):
                            src = glo[:, int(lo_off[t] - lo_off[t0]) + j2, :]
                            nc.tensor.matmul(ps[:], oh_at(k), src,
                                             start=(k == 0), stop=(k == nch - 1),
                                             skip_group_check=True)
                            k += 1
                        for j2 in range(int(C[t, 1])):
                            src = ghi[:, int(hi_off[t] - hi_off[t0]) + j2, :]
                            nc.tensor.matmul(ps[:], oh_at(k), src,
                                             start=(k == 0), stop=(k == nch - 1),
                                             skip_group_check=True)
                            k += 1

                        if is_conv1:
                            # hs = dis * relu(dis*(ps + g) [+ b1])
                            if has_b1:
                                u0 = wpool.tile([128, 128], f32, tag="u0")
                                nc.vector.scalar_tensor_tensor(
                                    u0[:], ps[:], dissb[:, t:t + 1],
                                    gkeep[:, t, :], mult, add)
                                u1 = wpool.tile([128, 128], f32, tag="u1")
                                nc.vector.tensor_tensor(u1[:], u0[:], b1sb[:], add)
                                hs = wpool.tile([128, 128], f16, tag="hs")
                                nc.scalar.activation(hs[:], u1[:], Relu,
                                                     scale=dissb[:, t:t + 1])
                            else:
                                u0 = wpool.tile([128, 128], f16, tag="u0")
                                nc.vector.scalar_tensor_tensor(
                                    u0[:], ps[:], dis2sb[:, t:t + 1],
                                    gkeep[:, t, :], mult, add)
                                hs = wpool.tile([128, 128], f16, tag="hs")
                                nc.scalar.activation(hs[:], u0[:], Relu)
                            pst = psT.tile([128, 128], f16, tag="psT")
                            nc.tensor.transpose(pst[:], hs[:], identsb[:])
                            hsT = wpool.tile([128, 128], f16, tag="hsT")
                            nc.scalar.copy(hsT[:], pst[:])
                            psh = psH.tile([128, 128], f32, tag="psH")
                            nc.tensor.matmul(psh[:], hsT[:], Wcsb[:],
                                             start=True, stop=True,
                                             skip_group_check=True)
                            hct = wpool.tile([128, 128], f16, tag="hct")
                            nc.scalar.copy(hct[:], psh[:])
                            nc.scalar.activation(hcdkeep[:, t, :], psh[:], Copy,
                                                 scale=dissb[:, t:t + 1])
                            if t < 32:
                                nc.sync.dma_start(
                                    hcsa[t * 128:(t + 1) * 128, :], hct[:])
                            else:
                                tt = t - 32
                                nc.sync.dma_start(
                                    hcsb[tt * 128:(tt + 1) * 128, :], hct[:])
                        else:
                            ot = wpool.tile([128, 128], f32, tag="ot")
                            nc.vector.scalar_tensor_tensor(
                                ot[:], ps[:], dissb[:, t:t + 1],
                                hcdkeep[:, t, :], mult, add)
                            if has_bc:
                                nc.vector.tensor_tensor(ot[:], ot[:], bcsb[:], add)
                            nc.sync.dma_start(
                                out_ml.ap()[t * 128:(t + 1) * 128, :], ot[:])
                    if is_conv1:
                        emit_ag([c for c in ag2_chunks if c[0] == 0],
                                hcsa, hcsb, hctab_lo, hctab_hi,
                                t1 * 128 if t1 <= 32 else LHL + (t1 - 32) * 128,
                                ag2_fired)

            conv_pass(ftab_lo, ftab_hi, idxl1sb, idxh1sb, True)
            # pre-emit conv2's first lo gathers (they only need hctab_lo) so
            # they stream during conv1's tail, before the gpsimd engine parks
            # on the AG2-hi trigger's input wait
            c2_emitters = make_conv(hctab_lo, hctab_hi, idxl1sb, idxh1sb,
                                    False)
            c2_emitters[0](0)
            c2_emitters[0](1)
            emit_ag(ag2_chunks, hcsa, hcsb, hctab_lo, hctab_hi, SH, ag2_fired)
            conv_pass(hctab_lo, hctab_hi, idxl1sb, idxh1sb, False,
                      emitters=c2_emitters)

    nc.compile()
    return nc


def kernel(x, edge_index, W1, b1, W_mu, b_mu, W_logstd, b_logstd):
    global LAST_RESULTS
    from concourse.bass_utils import run_bass_kernel_spmd

    x = np.asarray(x, dtype=np.float32)
    W1 = np.asarray(W1, dtype=np.float32)
    b1 = np.asarray(b1, dtype=np.float32)
    W_mu = np.asarray(W_mu, dtype=np.float32)
    b_mu = np.asarray(b_mu, dtype=np.float32)
    W_logstd = np.asarray(W_logstd, dtype=np.float32)
    b_logstd = np.asarray(b_logstd, dtype=np.float32)

    has_b1 = bool(np.any(b1 != 0.0))
    bc = np.concatenate([b_mu, b_logstd])
    has_bc = bool(np.any(bc != 0.0))

    ek = np.asarray(edge_index).tobytes()
    key = (ek[:64] + ek[-64:], len(ek), has_b1, has_bc)
    cached = _CACHE.get("k")
    if cached is not None and cached[0] == key:
        _, dis, core_data, meta, nc = cached
    else:
        dis, core_data, meta = _preprocess(edge_index)
        nc = _build_nc(meta, has_b1, has_bc)
        _CACHE["k"] = (key, dis, core_data, meta, nc)

    x2T = np.zeros((IN, NPAD), np.float16)
    x2T[:, :N] = (x * dis[:, None]).T.astype(np.float16)
    W1h = W1.astype(np.float16)
    Wch = np.concatenate([W_mu, W_logstd], axis=1).astype(np.float16)
    disP = np.zeros(NPAD, np.float32)
    disP[:N] = dis
    iota = np.tile(np.arange(128, dtype=np.float16)[None, :], (128, OHB))
    ident = np.eye(128, dtype=np.float16)

    in_maps = []
    for c in range(NCORES):
        idxs, destT = core_data[c]
        disSh = disP[c * SH:(c + 1) * SH].reshape(NT, 128).T  # [128, NT]
        im = {
            "xTs": np.ascontiguousarray(x2T[:, c * SH:(c + 1) * SH]),
            "W1d": W1h, "Wcd": Wch,
            "dis2Td": np.ascontiguousarray((disSh * disSh).astype(np.float32)),
            "disTd": np.ascontiguousarray(disSh.astype(np.float32)),
            "iotad": np.ascontiguousarray(iota),
            "identd": ident,
            "idxl1d": idxs[0], "idxh1d": idxs[1],
            "destTd": destT,
        }
        if has_b1:
            im["b1rd"] = np.tile(b1[None, :], (128, 1)).astype(np.float32)
        if has_bc:
            im["bcrd"] = np.tile(bc[None, :], (128, 1)).astype(np.float32)
        in_maps.append(im)

    res = run_bass_kernel_spmd(nc, in_maps, core_ids=list(range(NCORES)),
                               trace=TRACE)
    LAST_RESULTS = res
    full = np.concatenate([res.results[c]["out_ml"] for c in range(NCORES)],
                          axis=0)[:N]
    mu = np.ascontiguousarray(full[:, :OUT])
    logstd = np.ascontiguousarray(full[:, OUT:])
    return (mu, logstd)
